# revision 7
# baseline (speedup 1.0000x reference)
"""DetectionLayer decode kernel for Trainium2 (Bass/Tile), 8-core SPMD.

Computes, for inputs [N, 85] and anchors [N, 4] (N = 2,000,000):
    cond    = inputs[:, 5] > 0.5
    pred_yx = inputs[:, :2] * anchors[:, 2:4] + anchors[:, :2]
    pred_hw = exp(inputs[:, 2:4]) * anchors[:, 2:4]
    out     = where(cond, concat([pred_yx, pred_hw, inputs[:, 4:]]), 0)

Memory-bound: HBM traffic is halved by streaming inputs/anchors/outputs
as bfloat16 (max rel err ~0.9e-2, within the 2e-2 gate). The score
column is nudged on the host so `bf16(score) > 0.5` matches the f32
`score > 0.5` decision exactly.

The big masked passthrough runs on the vector engine as a bitwise AND
over int32-viewed bf16 pairs (mask {-1,0} per row) — half the DVE
elements of a bf16 multiply and bit-exact. Rows are 85 cols (odd), so
two consecutive rows are viewed as 85 int32 words and masked with two
ANDs: words 0:43 carry the even row (plus the odd row's col 0 in the
straddle word's high half — recomputed later anyway), words 43:85 carry
the odd row.

Sharding: row dimension split into 8 equal-shape overlapping windows
(window R rows, stride S; 7*S + R == N) so every core runs the same NEFF
on a 128*K-row-aligned shard with no host-side padding copies.
"""
import sys

sys.path.insert(0, "/opt/trn_rl_repo")

import ml_dtypes
import numpy as np

import concourse.bacc as bacc
import concourse.mybir as mybir
from concourse.bass_utils import run_bass_kernel_spmd
from concourse.tile import TileContext

N = 2_000_000
C = 85
PW = C            # int32 words per two-row pair (2*85*2B = 85 words)
WE = 43           # words carrying the even row (incl. straddle word)
N_CORES = 8
P = 128           # SBUF partitions
K = 82            # anchor rows per partition per tile
TILE_ROWS = P * K  # 10496
T = 24            # tiles per core
R = T * TILE_ROWS  # 251,904 rows per core window
S = 249_728        # window stride; 7*S + R == N
THR = 0.5

BF16 = np.dtype(ml_dtypes.bfloat16)
# smallest bf16 strictly greater than 0.5 — host nudge target for rows
# where f32 score > 0.5 but bf16 rounding lands on exactly 0.5
BF16_ABOVE_HALF = BF16.type(0.50390625)

assert 7 * S + R == N and S % P == 0 and S <= R

_NC_CACHE = None


def _build_module(n_tiles=T):
    rows = n_tiles * TILE_ROWS
    nc = bacc.Bacc("TRN2", target_bir_lowering=False, debug=False)
    inp = nc.dram_tensor("inputs", [rows, C], mybir.dt.bfloat16, kind="ExternalInput")
    anc = nc.dram_tensor("anchors", [rows, 4], mybir.dt.bfloat16, kind="ExternalInput")
    out = nc.dram_tensor("out", [rows, C], mybir.dt.bfloat16, kind="ExternalOutput")

    # Slab mapping: partition p owns rows [p*nt*K, (p+1)*nt*K); within the
    # slab, tile t covers rows t*K..(t+1)*K. Input/output DMAs are 128
    # descriptors of K*CP*2 contiguous bytes each; the anchors preload is
    # one fully contiguous run per partition.
    iv = inp.ap().rearrange("(p t g) c -> t p (g c)", p=P, g=K)  # [nt, 128, K*C]
    ov = out.ap().rearrange("(p t g) c -> t p (g c)", p=P, g=K)
    # All anchors resident in SBUF: [128, nt*K*4], tile t at cols [t*K*4,(t+1)*K*4)
    av_all = anc.ap().rearrange("(p t g) c -> p (t g c)", p=P, g=K)

    # anchor-preload chunk boundaries (in tiles): small first chunk so
    # tile 0's mask op unblocks quickly
    anc_cuts = [0, 2, 8, 16, n_tiles]

    with TileContext(nc) as tc:
        with tc.tile_pool(name="anc", bufs=1) as apool, \
             tc.tile_pool(name="inp", bufs=7) as ipool, \
             tc.tile_pool(name="outp", bufs=5) as opool, \
             tc.tile_pool(name="msk", bufs=2) as kpool, \
             tc.tile_pool(name="amp", bufs=2) as mpool, \
             tc.tile_pool(name="ehp", bufs=2) as epool, \
             tc.tile_pool(name="typ", bufs=2) as ypool:
            anc_all = apool.tile([P, n_tiles * K * 4], mybir.dt.bfloat16, tag="anc_all")
            # Preload anchors on the gpsimd ring in chunks so tile 0's mask
            # op doesn't wait for the whole transfer.
            for c_lo, c_hi in zip(anc_cuts[:-1], anc_cuts[1:]):
                c0, c1 = c_lo * K * 4, c_hi * K * 4
                nc.gpsimd.dma_start(out=anc_all[:, c0:c1], in_=av_all[:, c0:c1])
            for t in range(n_tiles):
                in_t = ipool.tile([P, K * C], mybir.dt.bfloat16, tag="in")
                out_t = opool.tile([P, K * C], mybir.dt.bfloat16, tag="out")
                mi_t = kpool.tile([P, K], mybir.dt.int32, tag="mi")
                am_t = mpool.tile([P, K * 4], mybir.dt.float32, tag="am")
                eh_t = epool.tile([P, K * 2], mybir.dt.float32, tag="eh")
                ty_t = ypool.tile([P, K * 2], mybir.dt.float32, tag="ty")

                (nc.sync if t % 2 == 0 else nc.scalar).dma_start(
                    out=in_t[:], in_=iv[t])

                ing = in_t[:].rearrange("p (g c) -> p g c", c=C)
                outg = out_t[:].rearrange("p (g c) -> p g c", c=C)
                inw = in_t[:].bitcast(mybir.dt.int32).rearrange(
                    "p (h w) -> p h w", w=PW)
                outw = out_t[:].bitcast(mybir.dt.int32).rearrange(
                    "p (h w) -> p h w", w=PW)
                ang = anc_all[:, t * K * 4:(t + 1) * K * 4].rearrange(
                    "p (g c) -> p g c", c=4)
                amg = am_t[:].rearrange("p (g c) -> p g c", c=4)
                ehg = eh_t[:].rearrange("p (g c) -> p g c", c=2)
                tyg = ty_t[:].rearrange("p (g c) -> p g c", c=2)
                score = ing[:, :, 5:6]

                # mask: -1 (all ones) where score > THR else 0
                nc.vector.tensor_scalar(
                    out=mi_t[:].unsqueeze(2),
                    in0=score,
                    scalar1=THR,
                    scalar2=-1.0,
                    op0=mybir.AluOpType.is_gt,
                    op1=mybir.AluOpType.mult,
                )
                # masked passthrough as int32 pairs, two rows = PW words:
                # words 0:WE get the even row's mask, words WE:PW the odd
                # row's (cols 0..3 recomputed below)
                mih = mi_t[:].rearrange("p (h two) -> p h two", two=2)
                nc.vector.tensor_tensor(
                    out=outw[:, :, 0:WE],
                    in0=inw[:, :, 0:WE],
                    in1=mih[:, :, 0:1].broadcast_to([P, K // 2, WE]),
                    op=mybir.AluOpType.bitwise_and,
                )
                nc.vector.tensor_tensor(
                    out=outw[:, :, WE:PW],
                    in0=inw[:, :, WE:PW],
                    in1=mih[:, :, 1:2].broadcast_to([P, K // 2, PW - WE]),
                    op=mybir.AluOpType.bitwise_and,
                )
                # masked anchors (f32): am = (score > THR) * anchors
                nc.vector.scalar_tensor_tensor(
                    out=amg,
                    in0=score.broadcast_to([P, K, 4]),
                    scalar=THR,
                    in1=ang,
                    op0=mybir.AluOpType.is_gt,
                    op1=mybir.AluOpType.mult,
                )
                # eh = exp(in[:, 2:4]) in f32 on the scalar engine
                nc.scalar.activation(
                    ehg,
                    ing[:, :, 2:4],
                    mybir.ActivationFunctionType.Exp,
                )
                # out[:, 2:4] = eh * am_hw   (f32 x f32 -> bf16)
                nc.vector.tensor_mul(outg[:, :, 2:4], ehg, amg[:, :, 2:4])
                # ty = in_yx * am_hw         (bf16 x f32 -> f32)
                nc.vector.tensor_mul(tyg, ing[:, :, 0:2], amg[:, :, 2:4])
                # out[:, 0:2] = ty + am_yx   (f32 x f32 -> bf16)
                nc.vector.tensor_add(outg[:, :, 0:2], tyg, amg[:, :, 0:2])

                (nc.scalar if t % 2 == 0 else nc.sync).dma_start(
                    out=ov[t], in_=out_t[:])
    nc.compile()
    return nc


def _get_module():
    global _NC_CACHE
    if _NC_CACHE is None:
        _NC_CACHE = _build_module()
    return _NC_CACHE


def _prep_bf16(inputs):
    """inputs f32 [N, C] -> bf16, with col 5 nudged so the
    device-side `bf16(score) > 0.5` reproduces `f32 score > 0.5`
    bit-exactly. (f32 values in (0.5, 0.5 + 2^-9) round DOWN to bf16
    0.5; bump them to the next representable bf16. Values <= 0.5 can
    never round above 0.5.)
    """
    xb = inputs.astype(BF16)
    s32 = inputs[:, 5]
    flipped = (s32 > THR) & (xb[:, 5].astype(np.float32) <= THR)
    if flipped.any():
        xb[flipped, 5] = BF16_ABOVE_HALF
    return xb


def _run(inputs, anchors, **spmd_kwargs):
    inputs = np.ascontiguousarray(np.asarray(inputs, dtype=np.float32))
    anchors = np.ascontiguousarray(np.asarray(anchors, dtype=np.float32))
    assert inputs.shape == (N, C) and anchors.shape == (N, 4)

    inputs_bf = _prep_bf16(inputs)
    anchors_bf = np.ascontiguousarray(anchors.astype(BF16))

    nc = _get_module()
    in_maps = [
        {"inputs": inputs_bf[i * S : i * S + R], "anchors": anchors_bf[i * S : i * S + R]}
        for i in range(N_CORES)
    ]
    res = run_bass_kernel_spmd(nc, in_maps, core_ids=list(range(N_CORES)), **spmd_kwargs)

    out = np.empty((N, C), dtype=np.float32)
    for i in range(N_CORES - 1):
        out[i * S : (i + 1) * S] = res.results[i]["out"][:S]
    out[(N_CORES - 1) * S :] = res.results[N_CORES - 1]["out"]
    return out, res


def kernel(inputs, anchors):
    out, _ = _run(inputs, anchors)
    return out


if __name__ == "__main__":
    rng = np.random.default_rng(0)
    x = rng.random((N, C), dtype=np.float32)
    a = rng.random((N, 4), dtype=np.float32)
    y = kernel(x, a)
    print("ran ok", y.shape, y.dtype)
